# revision 19
# baseline (speedup 1.0000x reference)
"""Multi-head causal attention (B=2, S=2048, D=1024, H=16, d=64) on 8 trn2 cores.

Sharding: core c -> batch b=c//4, head-group hg=c%4 (4 heads, 256 of 1024 dims).
Each core computes its 4 heads' attention + its partial out-projection; host
sums the 4 partials per batch and adds the bias.

All matmuls run in bf16 (fp32 PSUM accumulation). Attention uses the
transposed layout throughout: S^T[k,q] tiles, so softmax normalization is
deferred (ones-column appended to V gives denominators per query) and no P
transposes are needed. The causal mask is folded into the score matmul as one
extra identity x trimask accumulation on the diagonal tiles, and all diagonal
tiles are rectangularly trimmed to the causal width.
"""
import sys

sys.path.insert(0, "/opt/trn_rl_repo")

import numpy as np
import ml_dtypes
import concourse.bass as bass
import concourse.mybir as mybir
from concourse import bacc
from concourse.tile import TileContext
from concourse.bass_utils import run_bass_kernel_spmd

F32 = mybir.dt.float32
BF16 = mybir.dt.bfloat16
AF = mybir.ActivationFunctionType
OP = mybir.AluOpType

S = 2048          # sequence length
D = 1024          # model dim
HD = 64           # head dim
NHL = 4           # heads per core
DL = 256          # local out dims (NHL * HD)
NQC = 4           # q chunks of 512
QW = 512          # q chunk width
NKP = 16          # kp chunks of 128
NST = 16          # seq tiles of 128
NIC = 8           # input-dim chunks of 128
MASKVAL = -1.0e6


def build_bass():
    nc = bacc.Bacc("TRN2", target_bir_lowering=False, debug=False, num_devices=8)

    x_d = nc.dram_tensor("x", [S, D], BF16, kind="ExternalInput")
    wq_d = nc.dram_tensor("wq", [D, DL], BF16, kind="ExternalInput")
    wk_d = nc.dram_tensor("wk", [D, DL], BF16, kind="ExternalInput")
    wv_d = nc.dram_tensor("wv", [D, DL], BF16, kind="ExternalInput")
    wo_d = nc.dram_tensor("wo", [DL, D], BF16, kind="ExternalInput")
    mb_d = nc.dram_tensor("maskb", [128, QW], BF16, kind="ExternalInput")
    vo_d = nc.dram_tensor("vones", [128, NHL, 1], BF16, kind="ExternalInput")
    out_d = nc.dram_tensor("out", [S, D], F32, kind="ExternalOutput")

    with TileContext(nc) as tc:
        with (
            tc.tile_pool(name="consts", bufs=1) as consts,
            tc.tile_pool(name="xt", bufs=1) as xtp,
            tc.tile_pool(name="qk", bufs=1) as qkp,
            tc.tile_pool(name="vv", bufs=1) as vvp,
            tc.tile_pool(name="ctxn", bufs=1) as ctxnp,
            tc.tile_pool(name="ptp", bufs=8) as ptp,
            tc.tile_pool(name="rbp", bufs=4) as rbp,
            tc.tile_pool(name="outp", bufs=4) as outp,
            tc.tile_pool(name="psST", bufs=2, space="PSUM") as psST,
            tc.tile_pool(name="psCTX", bufs=2, space="PSUM") as psCTX,
            tc.tile_pool(name="psOUT", bufs=2, space="PSUM") as psOUT,
        ):
            # ---- inputs: wq/wk first, then x -> XT via DMA transpose (one
            # tile per chunk so consumers wait per-chunk), then the rest.
            # Single queue, transposes contiguous: 2 xbar-mode transitions.
            wq = consts.tile([128, NIC, DL], BF16, tag="wq")
            wk = consts.tile([128, NIC, DL], BF16, tag="wk")
            wv = consts.tile([128, NIC, DL], BF16, tag="wv")
            wo = consts.tile([128, 2, D], BF16, tag="wo")
            maskb = consts.tile([128, QW], BF16, tag="maskb")
            vext = vvp.tile([128, NKP, NHL, HD + 1], BF16, tag="vext")

            nc.sync.dma_start(out=wq, in_=wq_d.ap().rearrange("(c p) n -> p c n", p=128))
            nc.sync.dma_start(out=wk, in_=wk_d.ap().rearrange("(c p) n -> p c n", p=128))

            xt_t = xtp.tile([128, NIC, S], BF16, tag="xt", name="xt")
            nc.sync.dma_start_transpose(xt_t[:, 0 : NIC // 2, :], x_d.ap()[:, 0 : D // 2])
            nc.scalar.dma_start_transpose(xt_t[:, NIC // 2 :, :], x_d.ap()[:, D // 2 :])
            xt = [xt_t[:, ic, :] for ic in range(NIC)]

            nc.scalar.dma_start(out=wv, in_=wv_d.ap().rearrange("(c p) n -> p c n", p=128))
            nc.scalar.dma_start(out=wo, in_=wo_d.ap().rearrange("(c p) n -> p c n", p=128))
            nc.scalar.dma_start(out=maskb, in_=mb_d.ap())
            # V_ext col 64 = 1.0
            for st in range(NST):
                nc.scalar.dma_start(out=vext[:, st, :, HD : HD + 1], in_=vo_d.ap())

            # ---- phase B: QKV projections ------------------------------------
            qt = qkp.tile([128, 2, S], BF16, tag="qt")
            kt = qkp.tile([128, 2, S], BF16, tag="kt")
            for p in range(2):
                for qc in range(NQC):
                    for dst, w in ((qt, wq), (kt, wk)):
                        acc_t = psST.tile([128, 2, QW], F32, tag="st", name="accqk")
                        acc = acc_t[:, 0, :]
                        for ic in range(NIC):
                            nc.tensor.matmul(
                                acc,
                                w[:, ic, 128 * p : 128 * (p + 1)],
                                xt[ic][:, qc * QW : (qc + 1) * QW],
                                start=(ic == 0),
                                stop=(ic == NIC - 1),
                            )
                        nc.vector.tensor_copy(dst[:, p, qc * QW : (qc + 1) * QW], acc)
            for st in range(NST):
                acc_t = psST.tile([128, 2, QW], F32, tag="st", name="accv")
                acc = acc_t[:, 0, 0:DL]
                for ic in range(NIC):
                    nc.tensor.matmul(
                        acc,
                        xt[ic][:, st * 128 : (st + 1) * 128],
                        wv[:, ic, :],
                        start=(ic == 0),
                        stop=(ic == NIC - 1),
                    )
                nc.vector.tensor_copy(
                    vext[:, st, :, 0:HD], acc.rearrange("p (h e) -> p h e", h=NHL)
                )

            # ---- phase C+D: attention + out-projection -----------------------
            ctxn = ctxnp.tile([128, 2, S], BF16, tag="ctxn")
            for qc in range(NQC):
                qsl = slice(qc * QW, (qc + 1) * QW)
                n_kp = 4 * qc + 4
                for p in range(2):
                    ctxa = psCTX.tile([HD + 1, QW], F32, tag="ctx")
                    ctxb = psCTX.tile([HD + 1, QW], F32, tag="ctx")
                    # bulk kp chunks (fully below the diagonal), 2 at a time;
                    # one st slot per head covering both kps of the group
                    for g in range(2 * qc):
                        sta = psST.tile([128, 2, QW], F32, tag="st")
                        stb = psST.tile([128, 2, QW], F32, tag="st")
                        for kig in range(2):
                            kp = 2 * g + kig
                            ksl = slice(kp * 128, (kp + 1) * 128)
                            for st_t, lo in ((sta, 0), (stb, 64)):
                                nc.tensor.matmul(
                                    st_t[:, kig, :],
                                    kt[lo : lo + 64, p, ksl],
                                    qt[lo : lo + 64, p, qsl],
                                    start=True,
                                    stop=True,
                                    tile_position=(lo, 0),
                                )
                        pta = ptp.tile([128, 2, QW], BF16, tag="pt")
                        ptb = ptp.tile([128, 2, QW], BF16, tag="pt")
                        nc.scalar.activation(pta, sta, AF.Exp, scale=0.125)
                        nc.scalar.activation(ptb, stb, AF.Exp, scale=0.125)
                        for kig in range(2):
                            kp = 2 * g + kig
                            nc.tensor.matmul(
                                ctxa, vext[:, kp, 2 * p, :], pta[:, kig, :],
                                start=(kp == 0), stop=False,
                            )
                            nc.tensor.matmul(
                                ctxb, vext[:, kp, 2 * p + 1, :], ptb[:, kig, :],
                                start=(kp == 0), stop=False,
                            )
                    # diagonal kp chunks, trimmed to causal width; one st slot
                    # per kp holds head A in lane 0, head B in lane 1
                    for j in range(4):
                        kp = 4 * qc + j
                        w = QW - 128 * j
                        ksl = slice(kp * 128, (kp + 1) * 128)
                        qtr = slice(qc * QW + 128 * j, (qc + 1) * QW)
                        st_ps = psST.tile([128, 2, QW], F32, tag="st")
                        pt = ptp.tile([128, 2, QW], BF16, tag="pt")
                        for h01, lo in ((0, 0), (1, 64)):
                            nc.tensor.matmul(
                                st_ps[:, h01, 0:w],
                                kt[lo : lo + 64, p, ksl],
                                qt[lo : lo + 64, p, qtr],
                                start=True,
                                stop=True,
                                tile_position=(lo, 0),
                            )
                        for h01 in range(2):
                            nc.scalar.activation(
                                pt[:, h01, 0:w], st_ps[:, h01, 0:w],
                                AF.Exp, scale=0.125,
                            )
                            nc.vector.tensor_mul(
                                pt[:, h01, 0:w], pt[:, h01, 0:w], maskb[:, 0:w]
                            )
                        for h01, ctx_t in ((0, ctxa), (1, ctxb)):
                            nc.tensor.matmul(
                                ctx_t[:, 128 * j : QW],
                                vext[:, kp, 2 * p + h01, :],
                                pt[:, h01, 0:w],
                                start=(kp == 0),
                                stop=(kp == n_kp - 1),
                            )
                    # normalize: ctxn[d, q] = ctx[d, q] / ctx[64, q]
                    for ctx_t, lo in ((ctxa, 0), (ctxb, 64)):
                        dcp = rbp.tile([1, QW], F32, tag="rec")
                        nc.vector.tensor_copy(dcp, ctx_t[HD : HD + 1, :])
                        rb = rbp.tile([HD, QW], F32, tag="rb")
                        nc.gpsimd.partition_broadcast(rb, dcp)
                        rec = rbp.tile([HD, QW], F32, tag="rb2")
                        nc.vector.reciprocal_approx_fast(rec, rb)
                        nc.vector.scalar_tensor_tensor(
                            out=ctxn[lo : lo + HD, p, qsl],
                            in0=ctx_t[0:HD, :],
                            scalar=1.0,
                            in1=rec,
                            op0=OP.mult,
                            op1=OP.mult,
                        )
                # ---- out-projection for this q-chunk
                for t in range(qc * 4, qc * 4 + 4):
                    tsl = slice(t * 128, (t + 1) * 128)
                    osb = outp.tile([128, D], F32, tag="osb")
                    for nh in range(2):
                        po = psOUT.tile([128, QW], F32, tag="po")
                        nsl = slice(nh * QW, (nh + 1) * QW)
                        nc.tensor.matmul(
                            po, ctxn[:, 0, tsl], wo[:, 0, nsl],
                            start=True, stop=False,
                        )
                        nc.tensor.matmul(
                            po, ctxn[:, 1, tsl], wo[:, 1, nsl],
                            start=False, stop=True,
                        )
                        nc.vector.tensor_copy(osb[:, nsl], po)
                    nc.sync.dma_start(out=out_d.ap()[tsl, :], in_=osb)

    nc.finalize()
    return nc


_VONES = np.ones((128, NHL, 1), dtype=ml_dtypes.bfloat16)


def _maskb():
    # multiplicative causal mask: 0 where q_local < kp_local, else 1
    m = np.ones((128, QW), dtype=np.float32)
    kp = np.arange(128)[:, None]
    q = np.arange(QW)[None, :]
    m[q < kp] = 0.0
    return m.astype(ml_dtypes.bfloat16)


def shard_inputs(x, Wq, Wk, Wv, Wo):
    x = np.asarray(x, dtype=ml_dtypes.bfloat16)
    Wq = np.asarray(Wq, dtype=ml_dtypes.bfloat16)
    Wk = np.asarray(Wk, dtype=ml_dtypes.bfloat16)
    Wv = np.asarray(Wv, dtype=ml_dtypes.bfloat16)
    Wo = np.asarray(Wo, dtype=ml_dtypes.bfloat16)
    mb = _maskb()
    in_maps = []
    for c in range(8):
        b, hg = divmod(c, 4)
        sl = slice(DL * hg, DL * (hg + 1))
        in_maps.append({
            "x": np.ascontiguousarray(x[b]),
            "wq": np.ascontiguousarray(Wq[:, sl]),
            "wk": np.ascontiguousarray(Wk[:, sl]),
            "wv": np.ascontiguousarray(Wv[:, sl]),
            "wo": np.ascontiguousarray(Wo[sl, :]),
            "maskb": mb,
            "vones": _VONES,
        })
    return in_maps


def run(inputs, trace=False, **kwargs):
    """Build, run on 8 cores, and return (full_output, BassKernelResults)."""
    nc = build_bass()
    bo = np.asarray(inputs["bo"], dtype=np.float32)
    in_maps = shard_inputs(**{k: v for k, v in inputs.items() if k != "bo"})
    res = run_bass_kernel_spmd(
        nc, in_maps, core_ids=list(range(8)), trace=trace, **kwargs
    )
    parts = [r["out"] for r in res.results]
    out = np.empty((2, S, D), dtype=np.float32)
    for b in range(2):
        out[b] = parts[4 * b] + parts[4 * b + 1] + parts[4 * b + 2] + parts[4 * b + 3]
        out[b] += bo[None, :]
    return out, res


def kernel(x, Wq, Wk, Wv, Wo, bo):
    out, _ = run(dict(x=x, Wq=Wq, Wk=Wk, Wv=Wv, Wo=Wo, bo=bo))
    return out


# revision 20
# speedup vs baseline: 1.0331x; 1.0331x over previous
"""Multi-head causal attention (B=2, S=2048, D=1024, H=16, d=64) on 8 trn2 cores.

Sharding: core c -> batch b=c//4, head-group hg=c%4 (4 heads, 256 of 1024 dims).
Each core computes its 4 heads' attention + its partial out-projection; host
sums the 4 partials per batch and adds the bias.

All matmuls run in bf16 (fp32 PSUM accumulation). Attention uses the
transposed layout throughout: S^T[k,q] tiles, so softmax normalization is
deferred (ones-column appended to V gives denominators per query) and no P
transposes are needed. The causal mask is folded into the score matmul as one
extra identity x trimask accumulation on the diagonal tiles, and all diagonal
tiles are rectangularly trimmed to the causal width.
"""
import sys

sys.path.insert(0, "/opt/trn_rl_repo")

import numpy as np
import ml_dtypes
import concourse.bass as bass
import concourse.mybir as mybir
from concourse import bacc
from concourse.tile import TileContext
from concourse.bass_utils import run_bass_kernel_spmd

F32 = mybir.dt.float32
BF16 = mybir.dt.bfloat16
AF = mybir.ActivationFunctionType
OP = mybir.AluOpType

S = 2048          # sequence length
D = 1024          # model dim
HD = 64           # head dim
NHL = 4           # heads per core
DL = 256          # local out dims (NHL * HD)
NQC = 4           # q chunks of 512
QW = 512          # q chunk width
NKP = 16          # kp chunks of 128
NST = 16          # seq tiles of 128
NIC = 8           # input-dim chunks of 128
MASKVAL = -1.0e6


def build_bass():
    nc = bacc.Bacc("TRN2", target_bir_lowering=False, debug=False, num_devices=8)

    x_d = nc.dram_tensor("x", [S, D], BF16, kind="ExternalInput")
    wq_d = nc.dram_tensor("wq", [D, DL], BF16, kind="ExternalInput")
    wk_d = nc.dram_tensor("wk", [D, DL], BF16, kind="ExternalInput")
    wv_d = nc.dram_tensor("wv", [D, DL], BF16, kind="ExternalInput")
    wo_d = nc.dram_tensor("wo", [DL, D], BF16, kind="ExternalInput")
    mb_d = nc.dram_tensor("maskb", [128, QW], BF16, kind="ExternalInput")
    vo_d = nc.dram_tensor("vones", [128, NHL, 1], BF16, kind="ExternalInput")
    out_d = nc.dram_tensor("out", [S, D], F32, kind="ExternalOutput")

    with TileContext(nc) as tc:
        with (
            tc.tile_pool(name="consts", bufs=1) as consts,
            tc.tile_pool(name="xt", bufs=1) as xtp,
            tc.tile_pool(name="qk", bufs=1) as qkp,
            tc.tile_pool(name="vv", bufs=1) as vvp,
            tc.tile_pool(name="ctxn", bufs=1) as ctxnp,
            tc.tile_pool(name="ptp", bufs=8) as ptp,
            tc.tile_pool(name="rbp", bufs=4) as rbp,
            tc.tile_pool(name="outp", bufs=4) as outp,
            tc.tile_pool(name="psST", bufs=2, space="PSUM") as psST,
            tc.tile_pool(name="psCTX", bufs=2, space="PSUM") as psCTX,
            tc.tile_pool(name="psOUT", bufs=2, space="PSUM") as psOUT,
        ):
            # ---- inputs: wq/wk first, then x -> XT via DMA transpose (one
            # tile per chunk so consumers wait per-chunk), then the rest.
            # Single queue, transposes contiguous: 2 xbar-mode transitions.
            wq = consts.tile([128, NIC, DL], BF16, tag="wq")
            wk = consts.tile([128, NIC, DL], BF16, tag="wk")
            wv = consts.tile([128, NIC, DL], BF16, tag="wv")
            wo = consts.tile([128, 2, D], BF16, tag="wo")
            maskb = consts.tile([128, QW], BF16, tag="maskb")
            vext = vvp.tile([128, NKP, NHL, HD + 1], BF16, tag="vext")

            nc.sync.dma_start(out=wq, in_=wq_d.ap().rearrange("(c p) n -> p c n", p=128))
            nc.sync.dma_start(out=wk, in_=wk_d.ap().rearrange("(c p) n -> p c n", p=128))

            xta = xtp.tile([128, NIC // 2, S], BF16, tag="xta", name="xta")
            xtb = xtp.tile([128, NIC // 2, S], BF16, tag="xtb", name="xtb")
            nc.sync.dma_start_transpose(xta, x_d.ap()[:, 0 : D // 2])
            nc.sync.dma_start_transpose(xtb, x_d.ap()[:, D // 2 :])
            xt = [xta[:, ic, :] for ic in range(NIC // 2)] + [
                xtb[:, ic, :] for ic in range(NIC // 2)
            ]

            nc.sync.dma_start(out=wv, in_=wv_d.ap().rearrange("(c p) n -> p c n", p=128))
            nc.sync.dma_start(out=wo, in_=wo_d.ap().rearrange("(c p) n -> p c n", p=128))
            nc.sync.dma_start(out=maskb, in_=mb_d.ap())
            # V_ext col 64 = 1.0
            for st in range(NST):
                nc.sync.dma_start(out=vext[:, st, :, HD : HD + 1], in_=vo_d.ap())

            # ---- phase B: QKV projections ------------------------------------
            qt = qkp.tile([128, 2, S], BF16, tag="qt")
            kt = qkp.tile([128, 2, S], BF16, tag="kt")
            for p in range(2):
                for qc in range(NQC):
                    for dst, w in ((qt, wq), (kt, wk)):
                        acc_t = psST.tile([128, 2, QW], F32, tag="st", name="accqk")
                        acc = acc_t[:, 0, :]
                        for ic in range(NIC):
                            nc.tensor.matmul(
                                acc,
                                w[:, ic, 128 * p : 128 * (p + 1)],
                                xt[ic][:, qc * QW : (qc + 1) * QW],
                                start=(ic == 0),
                                stop=(ic == NIC - 1),
                            )
                        nc.vector.tensor_copy(dst[:, p, qc * QW : (qc + 1) * QW], acc)
            for st in range(NST):
                acc_t = psST.tile([128, 2, QW], F32, tag="st", name="accv")
                acc = acc_t[:, 0, 0:DL]
                for ic in range(NIC):
                    nc.tensor.matmul(
                        acc,
                        xt[ic][:, st * 128 : (st + 1) * 128],
                        wv[:, ic, :],
                        start=(ic == 0),
                        stop=(ic == NIC - 1),
                    )
                nc.vector.tensor_copy(
                    vext[:, st, :, 0:HD], acc.rearrange("p (h e) -> p h e", h=NHL)
                )

            # ---- phase C+D: attention + out-projection -----------------------
            ctxn = ctxnp.tile([128, 2, S], BF16, tag="ctxn")
            for qc in range(NQC):
                qsl = slice(qc * QW, (qc + 1) * QW)
                n_kp = 4 * qc + 4
                for p in range(2):
                    ctxa = psCTX.tile([HD + 1, QW], F32, tag="ctx")
                    ctxb = psCTX.tile([HD + 1, QW], F32, tag="ctx")
                    # bulk kp chunks (fully below the diagonal), 2 at a time;
                    # one st slot per head covering both kps of the group
                    for g in range(2 * qc):
                        sta = psST.tile([128, 2, QW], F32, tag="st")
                        stb = psST.tile([128, 2, QW], F32, tag="st")
                        for kig in range(2):
                            kp = 2 * g + kig
                            ksl = slice(kp * 128, (kp + 1) * 128)
                            for st_t, lo in ((sta, 0), (stb, 64)):
                                nc.tensor.matmul(
                                    st_t[:, kig, :],
                                    kt[lo : lo + 64, p, ksl],
                                    qt[lo : lo + 64, p, qsl],
                                    start=True,
                                    stop=True,
                                    tile_position=(lo, 0),
                                )
                        pta = ptp.tile([128, 2, QW], BF16, tag="pt")
                        ptb = ptp.tile([128, 2, QW], BF16, tag="pt")
                        nc.scalar.activation(pta, sta, AF.Exp, scale=0.125)
                        nc.scalar.activation(ptb, stb, AF.Exp, scale=0.125)
                        for kig in range(2):
                            kp = 2 * g + kig
                            nc.tensor.matmul(
                                ctxa, vext[:, kp, 2 * p, :], pta[:, kig, :],
                                start=(kp == 0), stop=False,
                            )
                            nc.tensor.matmul(
                                ctxb, vext[:, kp, 2 * p + 1, :], ptb[:, kig, :],
                                start=(kp == 0), stop=False,
                            )
                    # diagonal kp chunks, trimmed to causal width; one st slot
                    # per kp holds head A in lane 0, head B in lane 1
                    for j in range(4):
                        kp = 4 * qc + j
                        w = QW - 128 * j
                        ksl = slice(kp * 128, (kp + 1) * 128)
                        qtr = slice(qc * QW + 128 * j, (qc + 1) * QW)
                        st_ps = psST.tile([128, 2, QW], F32, tag="st")
                        pt = ptp.tile([128, 2, QW], BF16, tag="pt")
                        for h01, lo in ((0, 0), (1, 64)):
                            nc.tensor.matmul(
                                st_ps[:, h01, 0:w],
                                kt[lo : lo + 64, p, ksl],
                                qt[lo : lo + 64, p, qtr],
                                start=True,
                                stop=True,
                                tile_position=(lo, 0),
                            )
                        for h01 in range(2):
                            nc.scalar.activation(
                                pt[:, h01, 0:w], st_ps[:, h01, 0:w],
                                AF.Exp, scale=0.125,
                            )
                            nc.vector.tensor_mul(
                                pt[:, h01, 0:w], pt[:, h01, 0:w], maskb[:, 0:w]
                            )
                        for h01, ctx_t in ((0, ctxa), (1, ctxb)):
                            nc.tensor.matmul(
                                ctx_t[:, 128 * j : QW],
                                vext[:, kp, 2 * p + h01, :],
                                pt[:, h01, 0:w],
                                start=(kp == 0),
                                stop=(kp == n_kp - 1),
                            )
                    # normalize: ctxn[d, q] = ctx[d, q] / ctx[64, q]
                    for ctx_t, lo in ((ctxa, 0), (ctxb, 64)):
                        dcp = rbp.tile([1, QW], F32, tag="rec")
                        nc.vector.tensor_copy(dcp, ctx_t[HD : HD + 1, :])
                        rb = rbp.tile([HD, QW], F32, tag="rb")
                        nc.gpsimd.partition_broadcast(rb, dcp)
                        rec = rbp.tile([HD, QW], F32, tag="rb2")
                        nc.vector.reciprocal_approx_fast(rec, rb)
                        nc.vector.scalar_tensor_tensor(
                            out=ctxn[lo : lo + HD, p, qsl],
                            in0=ctx_t[0:HD, :],
                            scalar=1.0,
                            in1=rec,
                            op0=OP.mult,
                            op1=OP.mult,
                        )
                # ---- out-projection for this q-chunk
                for t in range(qc * 4, qc * 4 + 4):
                    tsl = slice(t * 128, (t + 1) * 128)
                    osb = outp.tile([128, D], F32, tag="osb")
                    for nh in range(2):
                        po = psOUT.tile([128, QW], F32, tag="po")
                        nsl = slice(nh * QW, (nh + 1) * QW)
                        nc.tensor.matmul(
                            po, ctxn[:, 0, tsl], wo[:, 0, nsl],
                            start=True, stop=False,
                        )
                        nc.tensor.matmul(
                            po, ctxn[:, 1, tsl], wo[:, 1, nsl],
                            start=False, stop=True,
                        )
                        nc.vector.tensor_copy(osb[:, nsl], po)
                    nc.sync.dma_start(out=out_d.ap()[tsl, :], in_=osb)

    nc.finalize()
    return nc


_VONES = np.ones((128, NHL, 1), dtype=ml_dtypes.bfloat16)


def _maskb():
    # multiplicative causal mask: 0 where q_local < kp_local, else 1
    m = np.ones((128, QW), dtype=np.float32)
    kp = np.arange(128)[:, None]
    q = np.arange(QW)[None, :]
    m[q < kp] = 0.0
    return m.astype(ml_dtypes.bfloat16)


def shard_inputs(x, Wq, Wk, Wv, Wo):
    x = np.asarray(x, dtype=ml_dtypes.bfloat16)
    Wq = np.asarray(Wq, dtype=ml_dtypes.bfloat16)
    Wk = np.asarray(Wk, dtype=ml_dtypes.bfloat16)
    Wv = np.asarray(Wv, dtype=ml_dtypes.bfloat16)
    Wo = np.asarray(Wo, dtype=ml_dtypes.bfloat16)
    mb = _maskb()
    in_maps = []
    for c in range(8):
        b, hg = divmod(c, 4)
        sl = slice(DL * hg, DL * (hg + 1))
        in_maps.append({
            "x": np.ascontiguousarray(x[b]),
            "wq": np.ascontiguousarray(Wq[:, sl]),
            "wk": np.ascontiguousarray(Wk[:, sl]),
            "wv": np.ascontiguousarray(Wv[:, sl]),
            "wo": np.ascontiguousarray(Wo[sl, :]),
            "maskb": mb,
            "vones": _VONES,
        })
    return in_maps


def run(inputs, trace=False, **kwargs):
    """Build, run on 8 cores, and return (full_output, BassKernelResults)."""
    nc = build_bass()
    bo = np.asarray(inputs["bo"], dtype=np.float32)
    in_maps = shard_inputs(**{k: v for k, v in inputs.items() if k != "bo"})
    res = run_bass_kernel_spmd(
        nc, in_maps, core_ids=list(range(8)), trace=trace, **kwargs
    )
    parts = [r["out"] for r in res.results]
    out = np.empty((2, S, D), dtype=np.float32)
    for b in range(2):
        out[b] = parts[4 * b] + parts[4 * b + 1] + parts[4 * b + 2] + parts[4 * b + 3]
        out[b] += bo[None, :]
    return out, res


def kernel(x, Wq, Wk, Wv, Wo, bo):
    out, _ = run(dict(x=x, Wq=Wq, Wk=Wk, Wv=Wv, Wo=Wo, bo=bo))
    return out
